# revision 4
# baseline (speedup 1.0000x reference)
"""Cross-modal attention (audio queries image features) on 8 Trainium2 NeuronCores.

Problem (aq=1, sigmoid=0, softmax=1):
    q = aud @ Wq.T            [B, K, C]   B=16, K=64, C=1024, H=8 heads, D=128
    k = img @ Wk.T            [B, N, C]   N=4096
    v = img @ Wv.T
    attn = softmax(q_h @ k_h.T * D**-0.5)   per head
    out  = concat_h(attn @ v_h)             [B, K, C]

Kernel algebra (2x FLOP reduction vs naive):
    t_h  = q_h @ Wk_h                (tiny: contraction D=128)
    S    = T @ img.T                 (scores, contraction C)  <- no k materialization
    E    = exp(S * scale)            (unnormalized)
    U    = E @ img                   (contraction N)          <- no v materialization
    out_h= (U_h @ Wv_h.T) / rowsum(E)

Sharding: pure data-parallel over batch, 2 batches per core, no collectives.
Device dtypes: big matmuls in bf16 (fp32 PSUM accumulate), projections fp32.
"""

import threading

import numpy as np
import ml_dtypes

import concourse.bass as bass
import concourse.mybir as mybir
import concourse.tile as tile
from concourse import bacc
from concourse.bass import ds, ts
from concourse.bass_utils import run_bass_kernel_spmd
from concourse.masks import make_identity

F32 = mybir.dt.float32
BF16 = mybir.dt.bfloat16
BF = ml_dtypes.bfloat16

NCORES = 8
B, N, K, C, H = 16, 4096, 64, 1024, 8
D = C // H            # 128 head dim
BL = B // NCORES      # 2 batches per core
M = H * K             # 512 rows of T/S/U (head-major query rows)
SCALE = float(D) ** -0.5
NCH = 512             # kv tokens processed per chunk
NCHUNKS = N // NCH    # 8
NSUB = NCH // 128     # 4
CC = C // 128         # 8 channel chunks
MI = M // 128         # 4 row chunks of S


def _body(ctx, tc, img, audT, wqT, wk, wvT, out):
    nc = tc.nc
    AF = mybir.ActivationFunctionType
    AX = mybir.AxisListType

    wpool = ctx.enter_context(tc.tile_pool(name="weights", bufs=1))
    apool = ctx.enter_context(tc.tile_pool(name="aud", bufs=2))
    tpool = ctx.enter_context(tc.tile_pool(name="tt", bufs=2))
    ipool = ctx.enter_context(tc.tile_pool(name="imgT", bufs=2))
    npool = ctx.enter_context(tc.tile_pool(name="imgN", bufs=2))
    epool = ctx.enter_context(tc.tile_pool(name="escr", bufs=8))
    atpool = ctx.enter_context(tc.tile_pool(name="at", bufs=8))
    upool = ctx.enter_context(tc.tile_pool(name="ut", bufs=1))
    spool = ctx.enter_context(tc.tile_pool(name="stats", bufs=2))
    opool = ctx.enter_context(tc.tile_pool(name="osb", bufs=2))
    ps_a = ctx.enter_context(tc.tile_pool(name="ps_a", bufs=2, space="PSUM"))
    ps_t = ctx.enter_context(tc.tile_pool(name="ps_t", bufs=2, space="PSUM"))
    ps_u = ctx.enter_context(tc.tile_pool(name="ps_u", bufs=2, space="PSUM"))

    # ---- persistent weights ----
    w_q = wpool.tile([128, CC, C], F32, tag="wq")     # Wq.T tiled: [cin%128, cin//128, cout]
    nc.sync.dma_start(w_q[:], wqT.rearrange("(ci p) o -> p ci o", p=128))
    w_k = wpool.tile([128, CC, C], F32, tag="wk")     # Wk tiled: [dglob%128, dglob//128, cin]
    nc.sync.dma_start(w_k[:], wk.rearrange("(ho p) c -> p ho c", p=128))
    w_v = wpool.tile([128, CC, C], F32, tag="wv")     # Wv.T tiled: [c%128, c//128, dglob]
    nc.sync.dma_start(w_v[:], wvT.rearrange("(cc p) d -> p cc d", p=128))
    ident_f = wpool.tile([128, 128], F32, tag="identf")
    make_identity(nc, ident_f[:])
    ident_b = wpool.tile([128, 128], BF16, tag="identb")
    nc.vector.tensor_copy(ident_b[:], ident_f[:])

    for b in range(BL):
        # ---- stage A: qT = Wq @ aud.T  -> [C(out) x K], fp32 ----
        a_sb = apool.tile([128, CC, K], F32, tag="audT")
        nc.sync.dma_start(a_sb[:], audT[b].rearrange("(ci p) t -> p ci t", p=128))
        qT = apool.tile([128, CC, K], F32, tag="qT")
        for co in range(CC):
            ps = ps_a.tile([128, 512], F32, tag="ps_a")
            for ci in range(CC):
                nc.tensor.matmul(
                    ps[:, :K], w_q[:, ci, ts(co, 128)], a_sb[:, ci, :],
                    start=(ci == 0), stop=(ci == CC - 1),
                )
            nc.vector.tensor_copy(qT[:, co, :], ps[:, :K])

        # ---- stage B: T.T[c, h*K+t] = Wk_h.T @ q_h.T, cast bf16 ----
        TTb = tpool.tile([128, CC, M], BF16, tag="TT")
        for cc in range(CC):
            ps = ps_a.tile([128, 512], F32, tag="ps_a")
            for h in range(H):
                nc.tensor.matmul(
                    ps[:, ts(h, K)], w_k[:, h, ts(cc, 128)], qT[:, h, :],
                    start=True, stop=True,
                )
            nc.vector.tensor_copy(TTb[:, cc, :], ps[:])

        # ---- stage C: per kv-chunk: scores -> exp -> transpose -> U accumulate ----
        ut = upool.tile([128, CC, M], F32, tag="UT")        # U.T accumulator [c, m]
        sums = spool.tile([128, MI * NCHUNKS], F32, tag="sums")
        for j in range(NCHUNKS):
            # img.T chunk via xbar DMA transpose: [c, n] bf16
            ti = ipool.tile([128, CC, NCH], BF16, tag="imgT")
            for cc in range(CC):
                nc.sync.dma_start(
                    ti[:, cc, :], img[b, ds(j * NCH, NCH), ds(cc * 128, 128)],
                    transpose=True,
                )
            # img chunk natural: [n, c] bf16 (lhsT blocks for U)
            ninat = npool.tile([128, NSUB, C], BF16, tag="imgN")
            nc.sync.dma_start(
                ninat[:], img[b, ds(j * NCH, NCH), :].rearrange("(s p) c -> p s c", p=128)
            )

            # scores S[mi] = sum_cc TT[:,cc,mi].T @ imgT[:,cc,:]; exp fused w/ row-sum
            e_tiles = []
            for mi in range(MI):
                ps = ps_a.tile([128, 512], F32, tag="ps_a")
                for cc in range(CC):
                    nc.tensor.matmul(
                        ps[:], TTb[:, cc, ts(mi, 128)], ti[:, cc, :],
                        start=(cc == 0), stop=(cc == CC - 1),
                    )
                e = epool.tile([128, NCH], BF16, tag="E")
                nc.scalar.activation(
                    e[:], ps[:], AF.Exp, scale=SCALE,
                    accum_out=sums[:, mi * NCHUNKS + j: mi * NCHUNKS + j + 1],
                )
                e_tiles.append(e)

            # A.T tiles: transpose 128x128 blocks of E on TensorE
            at_tiles = []
            for ni in range(NSUB):
                pst = ps_t.tile([128, 512], BF16, tag="ps_t")
                for mi in range(MI):
                    nc.tensor.transpose(
                        pst[:, ts(mi, 128)], e_tiles[mi][:, ts(ni, 128)], ident_b[:]
                    )
                at = atpool.tile([128, M], BF16, tag="AT")
                nc.vector.tensor_copy(at[:], pst[:])
                at_tiles.append(at)

            # U.T += img_chunk.T @ A.T  (contraction over n within chunk in PSUM)
            for cc in range(CC):
                psu = ps_u.tile([128, 512], F32, tag="ps_u")
                for ni in range(NSUB):
                    nc.tensor.matmul(
                        psu[:], ninat[:, ni, ts(cc, 128)], at_tiles[ni][:],
                        start=(ni == 0), stop=(ni == NSUB - 1),
                    )
                if j == 0:
                    nc.vector.tensor_copy(ut[:, cc, :], psu[:])
                else:
                    nc.vector.tensor_add(ut[:, cc, :], ut[:, cc, :], psu[:])

        # ---- stage D: softmax denominators -> reciprocal, relayout to [tok, h] ----
        recips = spool.tile([128, MI], F32, tag="recip")
        for mi in range(MI):
            ssum = spool.tile([128, 1], F32, tag="ssum")
            nc.vector.reduce_sum(ssum[:], sums[:, ts(mi, NCHUNKS)], axis=AX.X)
            nc.vector.reciprocal(recips[:, mi: mi + 1], ssum[:])
        # recips[p, mi] = 1/sum for row m = mi*128+p = h*64+tok
        # -> recip_r[tok, h]; relayout via a tiny DRAM bounce
        r_dram = nc.dram_tensor(f"rscratch{b}", [512], F32).ap()
        nc.gpsimd.dma_start(r_dram.rearrange("(p q) -> p q", q=MI), recips[:])
        recip_r = spool.tile([64, H], F32, tag="recip_r")
        # value(t, h) = r_dram[((h%2)*64 + t)*MI + h//2]
        src = r_dram.rearrange("(hp t h2) -> t h2 hp", hp=2, t=64)
        nc.gpsimd.dma_start(recip_r.rearrange("t (h2 hp) -> t h2 hp", hp=2), src)

        # ---- stage F: out_h.T = Wv_h @ U_h.T; transpose back; scale by 1/rowsum ----
        osb = opool.tile([64, C], F32, tag="osb")
        for h in range(H):
            pso = ps_a.tile([128, 512], F32, tag="ps_a")
            for cc in range(CC):
                nc.tensor.matmul(
                    pso[:, :K], w_v[:, cc, ts(h, 128)], ut[:, cc, ts(h, K)],
                    start=(cc == 0), stop=(cc == CC - 1),
                )
            ot = opool.tile([128, K], F32, tag="otmp")
            nc.vector.tensor_copy(ot[:], pso[:, :K])
            pst = ps_t.tile([128, 512], F32, tag="ps_t")
            nc.tensor.transpose(pst[:64, :128], ot[:], ident_f[:])
            nc.vector.tensor_scalar_mul(
                osb[:, ts(h, 128)], pst[:64, :128], recip_r[:, h: h + 1]
            )
        nc.sync.dma_start(out[b], osb[:])



def build_module():
    nc = bacc.Bacc("TRN2", target_bir_lowering=False, debug=False,
                   num_devices=NCORES)
    img = nc.dram_tensor("img", [BL, N, C], BF16, kind="ExternalInput").ap()
    audT = nc.dram_tensor("audT", [BL, C, K], F32, kind="ExternalInput").ap()
    wqT = nc.dram_tensor("wqT", [C, C], F32, kind="ExternalInput").ap()
    wk_t = nc.dram_tensor("wk", [C, C], F32, kind="ExternalInput").ap()
    wvT = nc.dram_tensor("wvT", [C, C], F32, kind="ExternalInput").ap()
    out = nc.dram_tensor("out", [BL, K, C], F32, kind="ExternalOutput").ap()
    from contextlib import ExitStack
    with tile.TileContext(nc) as tc, ExitStack() as ctx:
        _body(ctx, tc, img, audT, wqT, wk_t, wvT, out)
    nc.compile()
    return nc


_cache = threading.local()


def _get_module():
    nc = getattr(_cache, "nc", None)
    if nc is None:
        nc = build_module()
        _cache.nc = nc
    return nc


def make_in_maps(img_fea, aud_fea, Wq, Wk, Wv):
    img_fea = np.asarray(img_fea, dtype=np.float32)
    aud_fea = np.asarray(aud_fea, dtype=np.float32)
    wqT = np.ascontiguousarray(np.asarray(Wq, dtype=np.float32).T)
    wk = np.ascontiguousarray(np.asarray(Wk, dtype=np.float32))
    wvT = np.ascontiguousarray(np.asarray(Wv, dtype=np.float32).T)
    img_bf = img_fea.astype(BF)
    audT = np.ascontiguousarray(aud_fea.transpose(0, 2, 1))
    in_maps = []
    for i in range(NCORES):
        sl = slice(i * BL, (i + 1) * BL)
        in_maps.append({
            "img": np.ascontiguousarray(img_bf[sl]),
            "audT": audT[sl],
            "wqT": wqT,
            "wk": wk,
            "wvT": wvT,
        })
    return in_maps


def _run(in_maps, trace=False, **kw):
    nc = _get_module()
    return run_bass_kernel_spmd(nc, in_maps, list(range(NCORES)), trace=trace, **kw)


def _numpy_fallback(img_fea, aud_fea, Wq, Wk, Wv, aq, sigmoid, softmax):
    img_fea = np.asarray(img_fea, np.float32)
    aud_fea = np.asarray(aud_fea, np.float32)
    Wq, Wk, Wv = (np.asarray(w, np.float32) for w in (Wq, Wk, Wv))
    if aq:
        query, key, value = aud_fea, img_fea, img_fea
    else:
        query, key, value = img_fea, aud_fea, aud_fea
    b, nq, c = query.shape
    h = H
    d = c // h
    scale = d ** -0.5

    def split(x):
        bb, nn, _ = x.shape
        return x.reshape(bb, nn, h, d).transpose(0, 2, 1, 3)

    q = split(query @ Wq.T)
    k = split(key @ Wk.T)
    v = split(value @ Wv.T)
    attn = np.einsum("bhnd,bhmd->bhnm", q, k) * scale
    if sigmoid:
        attn = 1.0 / (1.0 + np.exp(-attn))
    if softmax:
        attn = attn - attn.max(-1, keepdims=True)
        attn = np.exp(attn)
        attn = attn / attn.sum(-1, keepdims=True)
    x = np.einsum("bhnm,bhmd->bhnd", attn, v)
    return x.transpose(0, 2, 1, 3).reshape(b, nq, c).astype(np.float32)


def kernel(img_fea, aud_fea, Wq, Wk, Wv, aq, sigmoid, softmax):
    aq_i, sig_i, sm_i = int(aq), int(sigmoid), int(softmax)
    if (aq_i, sig_i, sm_i) != (1, 0, 1):
        return _numpy_fallback(img_fea, aud_fea, Wq, Wk, Wv, aq_i, sig_i, sm_i)
    in_maps = make_in_maps(img_fea, aud_fea, Wq, Wk, Wv)
    res = _run(in_maps)
    return np.concatenate([res.results[i]["out"] for i in range(NCORES)], axis=0)
